# revision 1
# baseline (speedup 1.0000x reference)
"""Trainium2 Bass kernel for ConvReshapeBefore (im2col patch extraction).

Full problem: x (32, 64, 64, 64) f32 NHWC, kernel 3x3 stride 1 valid ->
out (62*62*32, 3, 3, 64) f32 where out[(r*62+c)*32 + b] = x[b, r:r+3, c:c+3, :].

Sharding: data-parallel over batch, 4 batches per core across 8 cores.
Per-core kernel is pure data movement:
  - stage the 4-batch shard in SBUF: partition = h + 64*(b%2),
    free = (b//2)*4096 + w*64 + ch   (128 partitions x 8192 f32 = 4 MiB)
  - 12 store DMAs (one per (b, kernel-row i)), each a 3-dim AP:
      src  SBUF [[8192, 62(r)], [64, 62(c)], [1, 192]]   (overlapping windows)
      dst  DRAM [[142848, 62(r)], [2304, 62(c)], [1, 192]]
    -> 768B descriptors, writes each output element exactly once.
HBM traffic per core: 4 MiB read + 35.4 MiB write (vs 70.8 MiB for HBM->HBM).
"""

import numpy as np

import concourse.bass as bass
import concourse.mybir as mybir
from concourse.ap import AP
from concourse.bass_utils import run_bass_kernel_spmd

# Full-problem constants (hardcoded per harness contract)
B, H, W, C = 32, 64, 64, 64
K = 3
R = H - K + 1  # 62 output rows = output cols
NCORES = 8
BS = B // NCORES  # 4 batches per core

WC = W * C              # 4096 elements per (b, h) row
ROW = 2 * WC            # 8192 f32 free elements per SBUF partition
CHUNK = K * C           # 192-element contiguous descriptor
OUT_STRIDE_R = R * BS * K * K * C   # 142848
OUT_STRIDE_C = BS * K * K * C       # 2304
OUT_STRIDE_B = K * K * C            # 576


def _build_nc() -> bass.Bass:
    nc = bass.Bass(target_bir_lowering=False)
    x = nc.dram_tensor("x", [BS, H, W, C], mybir.dt.float32, kind="ExternalInput")
    out = nc.dram_tensor(
        "out", [R * R * BS, K, K, C], mybir.dt.float32, kind="ExternalOutput"
    )

    def load_ap(b):
        src = AP(x, b * H * WC, [[WC, H], [1, WC]])
        dst = AP(t, (H * (b % 2)) * ROW + (b // 2) * WC, [[ROW, H], [1, WC]])
        return dst, src

    def store_ap(b, i):
        src = AP(
            t,
            (i + H * (b % 2)) * ROW + (b // 2) * WC,
            [[ROW, R], [C, R], [1, CHUNK]],
        )
        dst = AP(
            out,
            b * OUT_STRIDE_B + i * CHUNK,
            [[OUT_STRIDE_R, R], [OUT_STRIDE_C, R], [1, CHUNK]],
        )
        return dst, src

    with (
        nc.sbuf_tensor("t", [128, ROW], mybir.dt.float32) as t,
        nc.semaphore("l0") as l0,
        nc.semaphore("l1") as l1,
        nc.semaphore("s0") as s0,
        nc.semaphore("s1") as s1,
        nc.Block() as block,
    ):
        # Two HWDGE rings (SP + ACT), each handling 2 batches, pipelined:
        # load b, then stores for b while the next load is in flight.
        @block.sync
        def _(sync):
            for n, b in enumerate((0, 1)):
                dst, src = load_ap(b)
                sync.dma_start(dst, src).then_inc(l0, 16)
            for n, b in enumerate((0, 1)):
                sync.wait_ge(l0, 16 * (n + 1))
                for i in range(K):
                    dst, src = store_ap(b, i)
                    sync.dma_start(dst, src).then_inc(s0, 16)
            sync.wait_ge(s0, 16 * 2 * K)

        @block.scalar
        def _(scalar):
            for n, b in enumerate((2, 3)):
                dst, src = load_ap(b)
                scalar.dma_start(dst, src).then_inc(l1, 16)
            for n, b in enumerate((2, 3)):
                scalar.wait_ge(l1, 16 * (n + 1))
                for i in range(K):
                    dst, src = store_ap(b, i)
                    scalar.dma_start(dst, src).then_inc(s1, 16)
            scalar.wait_ge(s1, 16 * 2 * K)

    return nc


_NC = None


def _get_nc():
    global _NC
    if _NC is None:
        _NC = _build_nc()
    return _NC


def kernel(x: np.ndarray, **_run_kwargs) -> np.ndarray:
    assert x.shape == (B, H, W, C), x.shape
    nc = _get_nc()
    x = np.ascontiguousarray(x, dtype=np.float32)
    in_maps = [{"x": x[d * BS : (d + 1) * BS]} for d in range(NCORES)]
    res = run_bass_kernel_spmd(nc, in_maps, list(range(NCORES)), **_run_kwargs)
    outs = [res.results[d]["out"].reshape(R * R, BS, K, K, C) for d in range(NCORES)]
    full = np.concatenate(outs, axis=1).reshape(R * R * B, K, K, C)
    if _run_kwargs:
        return full, res
    return full
